# revision 17
# baseline (speedup 1.0000x reference)
"""Trainium2 Bass kernel: 7x7 sliding-window self-similarity attention.

out[b,c,h,w] = sum_j softmax_j(x[h,w] * x[h+dh,w+dw]) * x[h+dh,w+dw]
over the 7x7 neighborhood (zero padding, pad=3).

Math (t-trick): only 25 canonical score fields s_d = x * shift(x, d) are
computed; mirror contributions are views: e_{-d}[i] = e_d[i-df], and with
t_d = s_d * e_d the numerator obeys sum_d x[i+d] e_d[i] = (sum of t views)
/ x[i], so out = psum_t / (x * psum_e). TensorE accumulates both view-sums
in PSUM via bf16 identity matmuls; exp runs on ACT; score/t mults on DVE
(bf16 packed 2x mode).

Pipelining:
- warm-up matmuls on a memset tile (no DMA dependency) open the PE HAM
  clock gate (1.2 -> 2.4 GHz needs ~3.4us of sustained PE activity) before
  the first real matmul; dummy ldweights bridge the inter-batch drain gap.
- software pipeline over the 50 (batch, field) pairs: score/exp emitted 2
  fields ahead; the t mult is emitted just-in-time before the field's
  matmuls so a stalled t never head-of-line-blocks the DVE queue (finals).
- batch 0's slab DMA is split into chunk-aligned pieces and fields 0-2
  (df<=3) are produced piecewise so the first matmuls start at ~quarter
  production latency.
- PREADD fields' mirror views are pre-merged on DVE (one add replaces four
  matmul chunk-streams) to balance PE (~167us) vs DVE (~160us) load.
- the last two fields of each batch are emitted chunk-major with finals
  right after each chunk's stop matmul, so drains stagger under the
  remaining chunks' streams and the next batch starts with a minimal PE
  gap (no mid-kernel HAM re-throttle).
- the very last chunk (last batch) runs as two 256-col halves so the
  closing den/recip/out chain is half-length and the first half's finals
  overlap the second half's matmul streams (shorter kernel tail).
Measured: ~181.6us HW exec on 8 cores (baseline fp32 kernel: 515-615us).

Layout: per core 32 images as 2 batches of 16; partition p = rowblock
(0..7)*16 + image(0..15); each partition holds a 22-row x 134-col
zero-padded bf16 slab (br=16 rows + 3 halo rows / 3 pad cols; wp=134).
The slab starts at tile element 1 so the output origin t0_off=406 is even
(bf16 pair alignment for DVE 2x mode). PSUM: 4 banks e-sum + 4 banks t-sum
per batch, per-chunk tiles.
"""

import numpy as np
import ml_dtypes

import concourse.bacc as bacc
import concourse.bass as bass  # noqa: F401
import concourse.tile as tile
from concourse import mybir
from concourse.bass_utils import run_bass_kernel_spmd

N_CORES = 8
F32 = mybir.dt.float32
BF16 = mybir.dt.bfloat16
NP_BF16 = ml_dtypes.bfloat16
MULT = mybir.AluOpType.mult

B, C, H, W = 4, 64, 128, 128
N_IMG_TOTAL = B * C
IMG_PER_CORE = N_IMG_TOTAL // N_CORES  # 32
N_BATCH = 2
IMG_PER_BATCH = IMG_PER_CORE // N_BATCH  # 16
RB_N = 8
RH = 3                        # row halo
CP = 3                        # col pad
MM_CHUNK = 512
N_WARM = 48                   # HAM warm-up matmuls
LEAD = 2                      # score/exp production lead (fields)
# canonical fields whose mirror views are pre-merged on DVE (one add
# replaces four matmul chunk-streams) to balance PE vs DVE load
PREADD = {(2, -3), (2, -1), (2, 1)}


def canonical_offsets():
    canon = [(0, dj) for dj in range(1, 4)]
    canon += [(di, dj) for di in range(1, 4) for dj in range(-3, 4)]
    canon += [(0, 0)]
    return canon


def view2d(ap, off, rows, cols, stride):
    """Strided [rows, cols] view at element offset `off` of a flat [P, L] AP."""
    a = ap.copy()
    pair_t = type(a.ap)
    part = list(a.ap)[0]
    a.ap = pair_t([list(part), [stride, rows], [1, cols]])
    a.offset = a.offset + off
    return a


def build_nc(h=H, w=W):
    br = h // RB_N               # 16
    wp = w + 2 * CP              # 134
    slab = br + 2 * RH           # 22
    P = IMG_PER_BATCH * RB_N     # 128

    nx = slab * wp               # 2948 per batch
    t0_off = RH * wp + CP + 1    # 406 (tile coord; +1 alignment shift)
    lb = (br - 1) * wp + w       # 2138 base run length
    le = t0_off + lb             # 2544 tile length for s/e/u
    lc = br * w                  # 2048 compact output per batch
    n_chunks = lc // MM_CHUNK    # 4
    rpc = MM_CHUNK // w          # 4 rows per chunk

    nc = bacc.Bacc("TRN2", target_bir_lowering=False, debug=False)
    x_in = nc.dram_tensor("x", [P, N_BATCH * nx], BF16, kind="ExternalInput")
    id_in = nc.dram_tensor("ident", [P, P], BF16, kind="ExternalInput")
    y_out = nc.dram_tensor("y", [P, N_BATCH * lc], BF16,
                           kind="ExternalOutput")

    canon = canonical_offsets()
    n_fields = len(canon)         # 25
    n_views = 2 * n_fields - 1    # 49 per psum field

    with tile.TileContext(nc) as tc:
        with (
            tc.tile_pool(name="big", bufs=1) as big,
            tc.tile_pool(name="sp", bufs=4) as spool,
            tc.tile_pool(name="ep", bufs=4) as epool,
            tc.tile_pool(name="up", bufs=4) as upool,
            tc.tile_pool(name="pp", bufs=2) as ppool,
            tc.tile_pool(name="fin", bufs=2) as fin,
            tc.tile_pool(name="ps", bufs=1, space="PSUM") as ps,
        ):
            pse = [ps.tile([P, MM_CHUNK], F32, tag="pse%d" % ci,
                          name="pse%d" % ci) for ci in range(n_chunks)]
            psu = [ps.tile([P, MM_CHUNK], F32, tag="psu%d" % ci,
                          name="psu%d" % ci) for ci in range(n_chunks)]

            # HAM warm-up: matmuls on a zeroed tile, no DMA dependency.
            # round-robin over all 8 psum banks so consecutive warm matmuls
            # never hit the same bank's WAW drain wait (211ns -> ~107ns each)
            wtile = big.tile([P, P], BF16, tag="warm")
            nc.vector.memset(wtile[:], 0)
            wbanks = [t for pair in zip(pse, psu) for t in pair]
            for wk in range(N_WARM):
                nc.tensor.matmul(wbanks[wk % 8][:, :P], wtile[:], wtile[:],
                                 start=True, stop=True)

            xts = []
            for bi in range(N_BATCH):
                xt = big.tile([P, 1 + nx], BF16, tag="x%d" % bi, name="xt")
                xts.append(xt)
            # split batch 0's slab DMA into chunk-aligned pieces so field 0
            # production starts as soon as the first rows land
            dma_cuts = [1, 1 + 8 * wp, 1 + 12 * wp, 1 + 16 * wp, 1 + nx]
            ident = big.tile([P, P], BF16, tag="id")
            nc.sync.dma_start(out=xts[0][:, dma_cuts[0]:dma_cuts[1]],
                              in_=x_in[:, dma_cuts[0] - 1:dma_cuts[1] - 1])
            nc.sync.dma_start(out=ident[:], in_=id_in[:])
            for pi in range(1, 4):
                nc.sync.dma_start(out=xts[0][:, dma_cuts[pi]:dma_cuts[pi + 1]],
                                  in_=x_in[:, dma_cuts[pi] - 1:
                                           dma_cuts[pi + 1] - 1])
            nc.sync.dma_start(out=xts[1][:, 1:], in_=x_in[:, nx:])
            xb = xts
            # production piece boundaries for fields with df<=6, aligned so
            # piece i covers matmul chunk i's views for both offsets
            def fcuts(df):
                return [t0_off - df] + [t0_off + (i + 1) * rpc * wp - df
                                        for i in range(n_chunks - 1)] + \
                       [t0_off + lb]

            fields = [(bi, k) for bi in range(N_BATCH)
                      for k in range(n_fields)]
            tiles = {}
            stash = {}

            def emit_se(x, s, e, df, a, b):
                """score+exp on subrange [a, b) of the run."""
                sv = s[:, a:b]
                if df == 0:
                    nc.scalar.activation(
                        out=sv, in_=x[:, a:b],
                        func=mybir.ActivationFunctionType.Square,
                    )
                else:
                    nc.vector.tensor_tensor(
                        out=sv, in0=x[:, a:b],
                        in1=x[:, a + df:b + df], op=MULT,
                    )
                nc.scalar.activation(
                    out=e[:, a:b], in_=sv,
                    func=mybir.ActivationFunctionType.Exp)

            def prod_se(bi, k, split=None):
                """score + exp for field k of batch bi."""
                x = xb[bi]
                di, dj = canon[k]
                df = di * wp + dj
                lo = t0_off - df
                ln = lb + df
                s = spool.tile([P, le], BF16, tag="s", name="s")
                e = epool.tile([P, le], BF16, tag="e", name="e")
                if split is None:
                    emit_se(x, s, e, df, lo, lo + ln)
                else:
                    emit_se(x, s, e, df, lo, split)
                tiles[(bi, k)] = (s, e, df, lo, ln)

            def finals(bi, ci):
                den = fin.tile([P, MM_CHUNK], F32, tag="den", name="den")
                r = fin.tile([P, MM_CHUNK], F32, tag="r", name="r")
                out_c = fin.tile([P, MM_CHUNK], BF16, tag="out",
                                 name="out")
                xc = view2d(xb[bi][:], t0_off + ci * rpc * wp, rpc, w, wp)
                nc.vector.tensor_tensor(out=den[:], in0=pse[ci][:],
                                        in1=xc, op=MULT)
                nc.vector.reciprocal_approx_fast(out=r[:], in_=den[:])
                nc.vector.tensor_tensor(out=out_c[:], in0=psu[ci][:],
                                        in1=r[:], op=MULT)
                o = bi * lc + ci * MM_CHUNK
                nc.sync.dma_start(out=y_out[:, o:o + MM_CHUNK], in_=out_c[:])

            def mm_pair(rec, ci, start, stop):
                s, e, u, df, lo, ln = rec
                offs = [t0_off] + ([t0_off - df] if df else [])
                for oi, to in enumerate(offs):
                    st = start and oi == 0
                    sp = stop and oi == len(offs) - 1
                    co = to + ci * rpc * wp
                    nc.tensor.matmul(pse[ci][:], ident[:],
                                     view2d(e[:], co, rpc, w, wp),
                                     start=st, stop=sp)
                    nc.tensor.matmul(psu[ci][:], ident[:],
                                     view2d(u[:], co, rpc, w, wp),
                                     start=st, stop=sp)

            def mm_half(rec, ci, h, stop):
                """half-chunk (2-row / 256-col) streams into a psum slice."""
                s, e, u, df, lo, ln = rec
                rh2 = rpc // 2
                hc = h * (MM_CHUNK // 2)
                offs = [t0_off] + ([t0_off - df] if df else [])
                for oi, to in enumerate(offs):
                    sp = stop and oi == len(offs) - 1
                    co = to + ci * rpc * wp + h * rh2 * wp
                    nc.tensor.matmul(pse[ci][:, hc:hc + MM_CHUNK // 2],
                                     ident[:], view2d(e[:], co, rh2, w, wp),
                                     start=False, stop=sp,
                                     skip_group_check=True)
                    nc.tensor.matmul(psu[ci][:, hc:hc + MM_CHUNK // 2],
                                     ident[:], view2d(u[:], co, rh2, w, wp),
                                     start=False, stop=sp,
                                     skip_group_check=True)

            def finals_half(bi, ci, h):
                HC = MM_CHUNK // 2
                den = fin.tile([P, HC], F32, tag="dh", name="den_h")
                r = fin.tile([P, HC], F32, tag="rh", name="r_h")
                out_c = fin.tile([P, HC], BF16, tag="oh", name="out_h")
                xc = view2d(xb[bi][:],
                            t0_off + ci * rpc * wp + h * (rpc // 2) * wp,
                            rpc // 2, w, wp)
                nc.vector.tensor_tensor(out=den[:],
                                        in0=pse[ci][:, h * HC:(h + 1) * HC],
                                        in1=xc, op=MULT)
                nc.vector.reciprocal_approx_fast(out=r[:], in_=den[:])
                nc.vector.tensor_tensor(out=out_c[:],
                                        in0=psu[ci][:, h * HC:(h + 1) * HC],
                                        in1=r[:], op=MULT)
                o = bi * lc + ci * MM_CHUNK + h * HC
                nc.sync.dma_start(out=y_out[:, o:o + HC], in_=out_c[:])

            def emit_t(s, e, u, a, b):
                nc.vector.tensor_tensor(out=u[:, a:b], in0=e[:, a:b],
                                        in1=s[:, a:b], op=MULT)

            def early_t(bi, k):
                """emit field k's t mult ahead of its step (tail de-stall)."""
                s, e, df, lo, ln = tiles[(bi, k)]
                u = upool.tile([P, le], BF16, tag="u", name="u")
                emit_t(s, e, u, lo, lo + ln)
                tiles[(bi, k)] = (s, e, df, lo, ln, u)

            def step(bi, k, split=None):
                """t mult + matmuls (+finals on the last field) for field k."""
                rec0 = tiles.pop((bi, k))
                u0 = rec0[5] if len(rec0) == 6 else None
                s, e, df, lo, ln = rec0[:5]
                u = u0 if u0 is not None else upool.tile([P, le], BF16,
                                                         tag="u", name="u")
                rec = (s, e, u, df, lo, ln)
                if split is not None:
                    # piecewise: emit score/exp/t per piece, then that
                    # chunk's matmuls; piece i covers chunk i's views
                    cuts = split
                    for ci in range(n_chunks):
                        if ci > 0:
                            emit_se(xb[bi], s, e, df, cuts[ci], cuts[ci + 1])
                        emit_t(s, e, u, cuts[ci], cuts[ci + 1])
                        mm_pair(rec, ci, start=(k == 0), stop=False)
                    return
                if u0 is None:
                    emit_t(s, e, u, lo, lo + ln)
                if canon[k] in PREADD:
                    # merge the two mirror views on DVE: one add replaces
                    # four matmul chunk-streams per psum field
                    pe_m = ppool.tile([P, lb], BF16, tag="pe", name="pe_m")
                    pu_m = ppool.tile([P, lb], BF16, tag="pu", name="pu_m")
                    ADD = mybir.AluOpType.add
                    nc.vector.tensor_tensor(
                        out=pe_m[:], in0=e[:, t0_off:t0_off + lb],
                        in1=e[:, t0_off - df:t0_off - df + lb], op=ADD)
                    nc.vector.tensor_tensor(
                        out=pu_m[:], in0=u[:, t0_off:t0_off + lb],
                        in1=u[:, t0_off - df:t0_off - df + lb], op=ADD)
                    for ci in range(n_chunks):
                        co = ci * rpc * wp
                        nc.tensor.matmul(pse[ci][:], ident[:],
                                         view2d(pe_m[:], co, rpc, w, wp),
                                         start=False, stop=False)
                        nc.tensor.matmul(psu[ci][:], ident[:],
                                         view2d(pu_m[:], co, rpc, w, wp),
                                         start=False, stop=False)
                    return
                if k == n_fields - 2:
                    stash[bi] = rec
                    return
                if k == n_fields - 1:
                    # chunk-major over the last two fields: each chunk stops
                    # and drains while later chunks still stream; the very
                    # last chunk runs as two 256-col halves so the final
                    # den/recip/out chain is half-length and overlaps the
                    # second half's streams
                    prev = stash.pop(bi)
                    for ci in range(n_chunks):
                        if bi == N_BATCH - 1 and ci == n_chunks - 1:
                            for hh in range(2):
                                mm_half(prev, ci, hh, stop=False)
                                mm_half(rec, ci, hh, stop=True)
                                finals_half(bi, ci, hh)
                            continue
                        mm_pair(prev, ci, start=False, stop=False)
                        mm_pair(rec, ci, start=False, stop=True)
                        finals(bi, ci)
                    if bi == 0:
                        # keep the PE activity window busy across the drain
                        for wk in range(16):
                            nc.tensor.ldweights(ident[:])
                    return
                for ci in range(n_chunks):
                    mm_pair(rec, ci, start=(k == 0), stop=False)

            def field_cuts(i):
                bi, k = fields[i]
                di, dj = canon[k]
                df = di * wp + dj
                if bi == 0 and k < 3 and 0 < df <= 6:
                    return fcuts(df)
                return None

            for j in range(LEAD):
                cj = field_cuts(j)
                prod_se(*fields[j], split=cj[1] if cj else None)
            for i in range(len(fields)):
                if i + LEAD < len(fields):
                    cp_ = field_cuts(i + LEAD)
                    prod_se(*fields[i + LEAD],
                            split=cp_[1] if cp_ else None)
                step(*fields[i], split=field_cuts(i))
                bi_i, k_i = fields[i]
                if k_i == n_fields - 3:
                    # emit the last two fields' t mults now so the
                    # chunk-major tail never waits on the DVE for them
                    early_t(bi_i, n_fields - 2)
                    early_t(bi_i, n_fields - 1)
    nc.compile()
    return nc


_NC_CACHE = {}


def _get_nc():
    if "nc" not in _NC_CACHE:
        _NC_CACHE["nc"] = build_nc()
    return _NC_CACHE["nc"]


def make_slabs(imgs, h=H, w=W):
    """[n,h,w] fp32 -> [n*RB_N, slab*wp] bf16 slab layout (p = rb*n + img)."""
    n = imgs.shape[0]
    br = h // RB_N
    slab = br + 2 * RH
    xp = np.pad(imgs, ((0, 0), (RH, RH), (CP, CP))).astype(NP_BF16)
    rows = (np.arange(RB_N) * br)[:, None] + np.arange(slab)
    sl = xp[:, rows, :]
    sl = sl.transpose(1, 0, 2, 3)
    return np.ascontiguousarray(sl.reshape(RB_N * n, -1))


def unslab_out(y, n_img, h=H, w=W):
    """[n*RB_N, br*w compact] -> [n, h, w]."""
    br = h // RB_N
    y = y.reshape(RB_N, n_img, br, w).transpose(1, 0, 2, 3)
    return np.ascontiguousarray(y.reshape(n_img, h, w))


def core_input(imgs32):
    """32 images fp32 -> [128, 2*nx] bf16 (two 16-image batches)."""
    parts = [make_slabs(imgs32[bi * IMG_PER_BATCH:(bi + 1) * IMG_PER_BATCH])
             for bi in range(N_BATCH)]
    return np.concatenate(parts, axis=1)


def core_output(y):
    """[128, 2*lc] fp32 -> [32, H, W]."""
    lc = (H // RB_N) * W
    parts = [unslab_out(y[:, bi * lc:(bi + 1) * lc], IMG_PER_BATCH)
             for bi in range(N_BATCH)]
    return np.concatenate(parts, axis=0)


def run(x, **spmd_kwargs):
    nc = _get_nc()
    imgs = np.ascontiguousarray(np.asarray(x).reshape(N_IMG_TOTAL, H, W))
    imgs = imgs.astype(np.float32, copy=False)
    ident = np.eye(128, dtype=NP_BF16)
    in_maps = [
        {"x": core_input(imgs[i * IMG_PER_CORE:(i + 1) * IMG_PER_CORE]),
         "ident": ident}
        for i in range(N_CORES)
    ]
    res = run_bass_kernel_spmd(nc, in_maps, core_ids=list(range(N_CORES)),
                               **spmd_kwargs)
    out = np.concatenate(
        [core_output(res.results[i]["y"]) for i in range(N_CORES)],
        axis=0,
    )
    return out.reshape(B, C, H, W).astype(np.float32, copy=False), res


def kernel(x):
    out, _ = run(x)
    return out

